# revision 23
# baseline (speedup 1.0000x reference)
"""Trainium2 Bass kernel for nn_DiffusionStar (retrieval_knn).

Computes eps_star = (x - sqrt(ab) * weighted_x) / sqrt(1 - ab) where
weighted_x is the softmax-weighted average of the train set under the
Gaussian kernel exp(-||x - sqrt(ab) x0||^2 / (2 (1 - ab))).

Two-stage retrieval design (the softmax is ~1-hot: at most ~9 rows per
query fall within 34 logits of the max, and mass below max-34 is <1e-15):

 - Device (8 cores, train sharded along N): stream a d-major fp8(e4m3)
   copy of the shard once -- 1 byte/element, the HBM-roofline cost --
   and emit the coarse cross rows c = fp8(coefA*x) . fp8(t) via
   DoubleRow fp8 matmuls. Screening noise from the two fp8
   quantizations is empirically |err| <= 47 logits (std 8.6).
 - Host: coarse_logit = c - coefB*(t_sq - D); every row within
   DELTA=170 of any query's coarse max (~400 rows total; the worst-case
   miss margin needs only 94) is rescored exactly in f64 and the
   softmax + weighted average is computed over those candidates only.

Engine budget per 512-row tile: one 1.57 MB DMA (alternating SP/Act HW
queues, which carry nothing else so their strict FIFOs never stall the
stream), 12 DoubleRow matmuls on PE (~2.9 us), one PSUM->SBUF fp16 copy
on DVE. The pass is DMA-bound at ~420 GB/s effective HBM bandwidth.
"""

import contextlib

import ml_dtypes
import numpy as np

from concourse import bacc, mybir, tile
from concourse import bass_utils

FP16 = mybir.dt.float16
FP8 = mybir.dt.float8e4
F32 = mybir.dt.float32
NP_FP8 = ml_dtypes.float8_e4m3

B = 32          # queries
D = 3072        # feature dim (c*h*w)
N = 100000      # train points
N_CORES = 8
N_SHARD = N // N_CORES          # 12500
TILE = 1250
TILE_PAD = 1280                 # SBUF free-dim stride (DoubleRow needs 16-aligned)
N_TILES = N_SHARD // TILE                # 10
N_PAD = N_TILES * TILE                   # 12500 (no padding)
MM_SLICES = [(0, 512), (512, 1024), (1024, TILE)]
KC = D // 128                            # 24 contraction chunks
KP = KC // 2                             # 12 DoubleRow chunk pairs
DELTA = 170.0                            # host candidate threshold
DR = mybir.MatmulPerfMode.DoubleRow


def build_nc(n_tiles=N_TILES, repeat=1, skip_compute=False, skip_dma=False):
    nc = bacc.Bacc("TRN2", target_bir_lowering=False, debug=False, num_devices=1)

    # d-major fp8 operand: [tile, p, pair, 2, n] so each SBUF partition's
    # bytes are one contiguous HBM run
    a8h = nc.dram_tensor(
        "a8h", [n_tiles, 128, KP, 2, TILE], FP8, kind="ExternalInput"
    ).ap()
    # x-side stationary (fp8 hi of coefA*x, transposed)
    xw8 = nc.dram_tensor("xw8", [128, KP, 2, B], FP8, kind="ExternalInput").ap()

    lg_out = nc.dram_tensor("lg_out", [B, n_tiles, TILE], FP16,
                            kind="ExternalOutput").ap()

    with tile.TileContext(nc) as tc, contextlib.ExitStack() as st:
        const = st.enter_context(tc.tile_pool(name="const", bufs=1))
        apool = st.enter_context(tc.tile_pool(name="apool", bufs=3))
        lpool = st.enter_context(tc.tile_pool(name="lpool", bufs=3))
        ps_c1 = st.enter_context(tc.tile_pool(name="ps_c1", bufs=2, space="PSUM"))

        xw8_s = const.tile([128, KP, 2, B], FP8)
        nc.sync.dma_start(xw8_s[:], xw8)

        def emit_tile(i):
            a8h_t = apool.tile([128, KP, 2, TILE_PAD], FP8, tag="a8h")
            # one big transfer per tile, alternating between the two HW DGE
            # queues (SP/Act); keep those engines free of non-DMA work so
            # their strict FIFOs never stall the DMA pipeline
            eng = nc.sync if (i % 2 == 0) else nc.scalar
            if not skip_dma:
                eng.dma_start(a8h_t[:, :, :, 0:TILE], a8h[i])
            if skip_compute:
                dmy = apool.tile([128, 1], F32, tag="dmy")
                nc.vector.reduce_max(dmy[:], a8h_t[:, 0, 0, 0:8],
                                     axis=mybir.AxisListType.X)
                return

            c1 = ps_c1.tile([B, TILE], F32, tag="c1")
            for j in range(KP):
                for c0, c1_ in MM_SLICES:
                    nc.tensor.matmul(
                        c1[:, c0:c1_], xw8_s[:, j], a8h_t[:, j, :, c0:c1_],
                        start=(j == 0), stop=(j == KP - 1), perf_mode=DR,
                    )
            lt = lpool.tile([B, TILE], FP16, tag="lt")
            nc.vector.tensor_copy(lt[:], c1[:])
            # stream results out on the Pool SWDGE queue (SP/Act untouched)
            nc.gpsimd.dma_start(lg_out[:, i], lt[:])

        def emit_pass():
            for i in range(n_tiles):
                emit_tile(i)

        # repeat>1 is a timing mode: loop the whole pass on-device so the
        # NEFF size stays constant and per-pass time can be measured by slope
        if repeat > 1:
            with tc.For_i(0, repeat):
                emit_pass()
        else:
            emit_pass()

    nc.compile()
    return nc


_NC_CACHE = {}


def _get_nc(n_tiles=N_TILES):
    if n_tiles not in _NC_CACHE:
        _NC_CACHE[n_tiles] = build_nc(n_tiles)
    return _NC_CACHE[n_tiles]


LAST_RESULT = None  # BassKernelResults of the most recent run (for test harness)
LAST_IN_MAPS = None  # per-core input dicts of the most recent run


def kernel(x, train_data, alphas_cumprod, t):
    x = np.asarray(x)
    train_data = np.asarray(train_data)
    alphas_cumprod = np.asarray(alphas_cumprod)
    t_idx = int(np.asarray(t))

    ab = float(alphas_cumprod[t_idx])
    s_ab = np.sqrt(ab)
    one_minus = 1.0 - ab
    coefA = s_ab / one_minus            # logits = coefA * (x . t) - coefB * t_sq
    coefB = ab / (2.0 * one_minus)
    inv = 1.0 / np.sqrt(one_minus)

    xf = x.reshape(B, D).astype(np.float64)
    xs = coefA * xf                      # fold coefA into the query side

    # x-side stationary operand (shared across cores), fp8 hi only
    x8h = xs.astype(NP_FP8)
    xw8 = np.zeros((KP, 128, 2, B), NP_FP8)
    for jp in range(KP):
        for r in range(2):
            sl = slice((2 * jp + r) * 128, (2 * jp + r + 1) * 128)
            xw8[jp, :, r, :] = x8h[:, sl].T
    xw8_dev = np.ascontiguousarray(xw8.transpose(1, 0, 2, 3))  # [128, KP, 2, B]

    tf = train_data.reshape(N, D)
    in_maps = []
    for c in range(N_CORES):
        shard = tf[c * N_SHARD : (c + 1) * N_SHARD].astype(np.float32)
        A_h8 = shard.T.astype(NP_FP8)                # [D, N_SHARD]
        # [tile, p, pair, 2, n] partition-major layout
        a8h_c = np.ascontiguousarray(
            A_h8.reshape(KP, 2, 128, N_TILES, TILE).transpose(3, 2, 0, 1, 4)
        )
        in_maps.append(dict(a8h=a8h_c, xw8=xw8_dev))

    nc = _get_nc()
    res = bass_utils.run_bass_kernel_spmd(nc, in_maps, core_ids=list(range(N_CORES)))
    global LAST_RESULT, LAST_IN_MAPS
    LAST_RESULT = res
    LAST_IN_MAPS = in_maps

    # coarse logits from device cross rows + host bias
    lg = np.stack(
        [r["lg_out"].reshape(B, N_PAD) for r in res.results]
    ).astype(np.float64)                                                  # [8,B,N_PAD]
    coarse_cross = np.concatenate(list(lg[:, :, :N_SHARD]), axis=1)       # [B, N]

    tf64 = tf.astype(np.float64)
    t_sq = np.einsum("nd,nd->n", tf64, tf64)
    bias = -coefB * (t_sq - float(D))
    Lc = coarse_cross + bias[None, :]

    mh = Lc.max(axis=1)
    cand = (Lc >= mh[:, None] - DELTA).any(axis=0)
    idx = np.nonzero(cand)[0]

    # exact rescore of candidates in f64
    sub = tf64[idx]                                  # [C, D]
    L_e = coefA * (xf @ sub.T) + bias[idx][None, :]  # [B, C]
    m_e = L_e.max(axis=1)
    P = np.exp(L_e - m_e[:, None])
    s_tot = P.sum(axis=1)
    weighted = (P @ sub) / s_tot[:, None]            # [B, D]

    out = inv * xf - (s_ab * inv) * weighted
    return out.reshape(x.shape).astype(np.float32)


# revision 24
# speedup vs baseline: 13.1597x; 13.1597x over previous
"""Trainium2 Bass kernel for nn_DiffusionStar (retrieval_knn).

Computes eps_star = (x - sqrt(ab) * weighted_x) / sqrt(1 - ab) where
weighted_x is the softmax-weighted average of the train set under the
Gaussian kernel exp(-||x - sqrt(ab) x0||^2 / (2 (1 - ab))).

Two-stage retrieval design (the softmax is ~1-hot: at most ~9 rows per
query fall within 34 logits of the max, and mass below max-34 is <1e-15):

 - Device (8 cores, train sharded along N): stream a d-major fp8(e4m3)
   copy of the shard once -- 1 byte/element, the HBM-roofline cost --
   and emit the coarse cross rows c = fp8(coefA*x) . fp8(t) via
   DoubleRow fp8 matmuls. Screening noise from the two fp8
   quantizations is empirically |err| <= 47 logits (std 8.6).
 - Host: coarse_logit = c - coefB*(t_sq - D); every row within
   DELTA=170 of any query's coarse max (~400 rows total; the worst-case
   miss margin needs only 94) is rescored exactly in f64 and the
   softmax + weighted average is computed over those candidates only.

Engine budget per 512-row tile: one 1.57 MB DMA (alternating SP/Act HW
queues, which carry nothing else so their strict FIFOs never stall the
stream), 12 DoubleRow matmuls on PE (~2.9 us), one PSUM->SBUF fp16 copy
on DVE. The pass is DMA-bound at ~420 GB/s effective HBM bandwidth.
"""

import contextlib

import ml_dtypes
import numpy as np

from concourse import bacc, mybir, tile
from concourse import bass_utils

FP16 = mybir.dt.float16
FP8 = mybir.dt.float8e4
F32 = mybir.dt.float32
NP_FP8 = ml_dtypes.float8_e4m3

B = 32          # queries
D = 3072        # feature dim (c*h*w)
N = 100000      # train points
N_CORES = 8
N_SHARD = N // N_CORES          # 12500
TILE = 512
N_TILES = (N_SHARD + TILE - 1) // TILE   # 25
N_PAD = N_TILES * TILE                   # 12800
KC = D // 128                            # 24 contraction chunks
KP = KC // 2                             # 12 DoubleRow chunk pairs
DELTA = 170.0                            # host candidate threshold
DR = mybir.MatmulPerfMode.DoubleRow


def build_nc(n_tiles=N_TILES, repeat=1, skip_compute=False, skip_dma=False):
    nc = bacc.Bacc("TRN2", target_bir_lowering=False, debug=False, num_devices=1)

    # d-major fp8 operand: [tile, p, pair, 2, n] so each SBUF partition's
    # bytes are one contiguous HBM run
    a8h = nc.dram_tensor(
        "a8h", [n_tiles, 128, KP, 2, TILE], FP8, kind="ExternalInput"
    ).ap()
    # x-side stationary (fp8 hi of coefA*x, transposed)
    xw8 = nc.dram_tensor("xw8", [128, KP, 2, B], FP8, kind="ExternalInput").ap()

    lg_out = nc.dram_tensor("lg_out", [B, n_tiles, TILE], FP16,
                            kind="ExternalOutput").ap()

    with tile.TileContext(nc) as tc, contextlib.ExitStack() as st:
        const = st.enter_context(tc.tile_pool(name="const", bufs=1))
        apool = st.enter_context(tc.tile_pool(name="apool", bufs=4))
        ps_c1 = st.enter_context(tc.tile_pool(name="ps_c1", bufs=2, space="PSUM"))

        xw8_s = const.tile([128, KP, 2, B], FP8)
        nc.sync.dma_start(xw8_s[:], xw8)
        logbuf = const.tile([B, n_tiles, TILE], FP16)

        def emit_tile(i):
            a8h_t = apool.tile([128, KP, 2, TILE], FP8, tag="a8h")
            # one big transfer per tile, alternating between the two HW DGE
            # queues (SP/Act); keep those engines free of non-DMA work so
            # their strict FIFOs never stall the DMA pipeline
            eng = nc.sync if (i % 2 == 0) else nc.scalar
            if not skip_dma:
                eng.dma_start(a8h_t[:], a8h[i])
            if skip_compute:
                dmy = apool.tile([128, 1], F32, tag="dmy")
                nc.vector.reduce_max(dmy[:], a8h_t[:, 0, 0, 0:8],
                                     axis=mybir.AxisListType.X)
                return

            c1 = ps_c1.tile([B, TILE], F32, tag="c1")
            for j in range(KP):
                nc.tensor.matmul(
                    c1[:], xw8_s[:, j], a8h_t[:, j],
                    start=(j == 0), stop=(j == KP - 1), perf_mode=DR,
                )
            nc.vector.tensor_copy(logbuf[:, i], c1[:])

        def emit_pass():
            for i in range(n_tiles):
                emit_tile(i)
            nc.sync.dma_start(lg_out, logbuf[:])

        # repeat>1 is a timing mode: loop the whole pass on-device so the
        # NEFF size stays constant and per-pass time can be measured by slope
        if repeat > 1:
            with tc.For_i(0, repeat):
                emit_pass()
        else:
            emit_pass()

    nc.compile()
    return nc


_NC_CACHE = {}


def _get_nc(n_tiles=N_TILES):
    if n_tiles not in _NC_CACHE:
        _NC_CACHE[n_tiles] = build_nc(n_tiles)
    return _NC_CACHE[n_tiles]


LAST_RESULT = None  # BassKernelResults of the most recent run (for test harness)
LAST_IN_MAPS = None  # per-core input dicts of the most recent run


def kernel(x, train_data, alphas_cumprod, t):
    x = np.asarray(x)
    train_data = np.asarray(train_data)
    alphas_cumprod = np.asarray(alphas_cumprod)
    t_idx = int(np.asarray(t))

    ab = float(alphas_cumprod[t_idx])
    s_ab = np.sqrt(ab)
    one_minus = 1.0 - ab
    coefA = s_ab / one_minus            # logits = coefA * (x . t) - coefB * t_sq
    coefB = ab / (2.0 * one_minus)
    inv = 1.0 / np.sqrt(one_minus)

    xf = x.reshape(B, D).astype(np.float64)
    xs = coefA * xf                      # fold coefA into the query side

    # x-side stationary operand (shared across cores), fp8 hi only
    x8h = xs.astype(NP_FP8)
    xw8 = np.zeros((KP, 128, 2, B), NP_FP8)
    for jp in range(KP):
        for r in range(2):
            sl = slice((2 * jp + r) * 128, (2 * jp + r + 1) * 128)
            xw8[jp, :, r, :] = x8h[:, sl].T
    xw8_dev = np.ascontiguousarray(xw8.transpose(1, 0, 2, 3))  # [128, KP, 2, B]

    tf = train_data.reshape(N, D)
    in_maps = []
    for c in range(N_CORES):
        shard = tf[c * N_SHARD : (c + 1) * N_SHARD].astype(np.float32)
        t_pad = np.zeros((N_PAD, D), np.float32)
        t_pad[:N_SHARD] = shard
        A_h8 = t_pad.T.astype(NP_FP8)                # [D, N_PAD]
        # [tile, p, pair, 2, n] partition-major layout
        a8h_c = np.ascontiguousarray(
            A_h8.reshape(KP, 2, 128, N_TILES, TILE).transpose(3, 2, 0, 1, 4)
        )
        in_maps.append(dict(a8h=a8h_c, xw8=xw8_dev))

    nc = _get_nc()
    res = bass_utils.run_bass_kernel_spmd(nc, in_maps, core_ids=list(range(N_CORES)))
    global LAST_RESULT, LAST_IN_MAPS
    LAST_RESULT = res
    LAST_IN_MAPS = in_maps

    # coarse logits from device cross rows + host bias
    lg = np.stack(
        [r["lg_out"].reshape(B, N_PAD) for r in res.results]
    ).astype(np.float64)                                                  # [8,B,N_PAD]
    coarse_cross = np.concatenate(list(lg[:, :, :N_SHARD]), axis=1)       # [B, N]

    tf64 = tf.astype(np.float64)
    t_sq = np.einsum("nd,nd->n", tf64, tf64)
    bias = -coefB * (t_sq - float(D))
    Lc = coarse_cross + bias[None, :]

    mh = Lc.max(axis=1)
    cand = (Lc >= mh[:, None] - DELTA).any(axis=0)
    idx = np.nonzero(cand)[0]

    # exact rescore of candidates in f64
    sub = tf64[idx]                                  # [C, D]
    L_e = coefA * (xf @ sub.T) + bias[idx][None, :]  # [B, C]
    m_e = L_e.max(axis=1)
    P = np.exp(L_e - m_e[:, None])
    s_tot = P.sum(axis=1)
    weighted = (P @ sub) / s_tot[:, None]            # [B, D]

    out = inv * xf - (s_ab * inv) * weighted
    return out.reshape(x.shape).astype(np.float32)
